# revision 9
# baseline (speedup 1.0000x reference)
"""Trainium2 Bass kernel for nn_CrossConvLayerV2 (gnn_message_passing).

Math (reference):
    coords = points[..., :3]; feats = points[..., 3:]          # [B,n,3], [B,n,f]
    probes[b,l,m] = centers[b,l] + PROBES[m]                    # [B,l,m,3]
    sq[b,l,m,n]  = ||coords[b,n] - probes[b,l,m]||^2
    kern         = C / (sq + C)          (C = 0.1)
    agg[b,l,m,f] = (1/n) sum_n kern * feats
    out[b,l,:]   = agg.reshape(l, m*f) @ W + bias               # [B,l,256]

Strategy (v2):
  - Shard centers dim l (256) over 8 cores -> 32 centers/core, zero
    communication; the host gathers the 8 [B,32,256] shards.
  - Per job (b, 16-center slab) and per 128-point chunk t:
      u = 8*(10*sq+1) via ONE K=24 bf16 matmul of "expanded" vectors
        (3x bf16 splits -> ~24-bit exact; see _prep helpers).
      kern = 1/u on the ACT engine (Reciprocal LUT), fp16.
      agg[(g,f), (m,l')] += feats_chunk^T @ kern^T  (PSUM accumulate)
      out = sum_m aggS_m^T @ W_rep_m  (bf16, single pass)
  - v2 performance structure (vs v1, 151.9us -> target ~65us):
      * 3-chunk PSUM supertiles (3 banks): ONE ACT reciprocal instruction
        per 3 chunks (FD=1248) amortizes the ~310-cycle ACT fixed cost.
      * matmul1 row-tiled: chunks t%3 -> PE row groups 0/32/64, three
        concurrent K=24 matmuls (c5/p5 replicated per row group).
      * matmul2 col-tiled: chunks t%4 -> PE col groups, M=16 outputs land
        on agg partitions 32g..32g+15; a K=1 zero-matmul opens the
        accumulation (clears has_written for the whole bank ONCE).
      * group-sum over g is folded into the weighter: W replicated at
        partition rows 32g+f (rows 32g+16..31 zero).
      * weighter is a single bf16 pass (error budget ~0.4% << 2e-2).
  - This walrus build encodes at most ONE semaphore wait per instruction;
    a post-build pass splits multi-wait instructions into single-wait
    NoOp carriers.
"""

import sys

sys.path.insert(0, "/opt/trn_rl_repo")

import numpy as np
import ml_dtypes

# ---- problem constants (hardcoded per contract) ----
B, N, L, D, F = 2, 4096, 256, 3, 16
M = 26
OUT_D = 256
COEFF = 0.1
DIST = 3.0
N_CORES = 8
L_LOC = L // N_CORES          # 32 centers per core
N_SLABS = 2                   # jobs per batch elem per core
L_SLAB = L_LOC // N_SLABS     # 16 centers per job
JM = M * L_SLAB               # 416 = free dim of kern^T tiles
N_JOBS = B * N_SLABS          # 4 jobs per core
NT = N // 128                 # 32 n-chunks
K5 = 24                       # expanded-distance contraction depth
NST = (NT + 2) // 3           # 11 supertiles per job (10x3 + 1x2)
SPL = 364                     # reciprocal column split: ACT [0:SPL], DVE rest
bf16 = ml_dtypes.bfloat16


def _make_probes() -> np.ndarray:
    angles = np.array(
        [[j * 0.125 - 0.125, i * 0.125 + (j - 1) * 0.0625] for j in range(3) for i in range(8)]
        + [[-0.25, 0.0], [0.25, 0.0]],
        dtype=np.float64,
    ) * (2.0 * np.pi)
    a, b = angles[:, 0], angles[:, 1]
    pts = np.stack([np.sin(a), np.cos(a) * np.cos(b), np.cos(a) * np.sin(b)], axis=-1) * DIST
    return pts.astype(np.float32)  # [26, 3]


PROBES = _make_probes()


def _split3_bf16(x):
    """x (f64) -> three bf16 arrays whose sum approximates x to ~24 bits."""
    x0 = x.astype(bf16)
    r1 = x - x0.astype(np.float64)
    x1 = r1.astype(bf16)
    x2 = (r1 - x1.astype(np.float64)).astype(bf16)
    return x0, x1, x2


_NC = None


def _act_reciprocal(nc, out_ap, in_ap):
    """nc.scalar.activation(func=Reciprocal) minus the library guard.
    out = 1/in_ on the ACT engine (LUT path; measured ~1.2e-5 rel here)."""
    import concourse.mybir as mybir

    eng = nc.scalar
    inputs = [eng.lower_ap(in_ap)]
    for val in (0.0, 1.0, 0.0):  # bias, scale, alpha — immediates
        inputs.append(mybir.ImmediateValue(dtype=mybir.dt.float32, value=val))
    return eng.add_instruction(
        mybir.InstActivation(
            name=nc.get_next_instruction_name(),
            func=mybir.ActivationFunctionType.Reciprocal,
            ins=inputs,
            outs=[eng.lower_ap(out_ap)],
        )
    )


def _split_multi_waits(nc):
    """This walrus build encodes at most ONE semaphore wait per instruction.
    Split every instruction with k>1 waits into (k-1) single-wait NoOps on
    the same engine immediately before it — identical blocking semantics."""
    import concourse.mybir as mybir

    n = 0
    for f in nc.m.functions:
        for bb in f.blocks:
            new_il = []
            for inst in bb.instructions:
                si = inst.sync_info
                waits = list(si.on_wait) if si is not None else []
                if len(waits) > 1:
                    for w in waits[:-1]:
                        nop = mybir.InstNoOp(name=f"{inst.name}-wsplit{n}", ins=[], outs=[])
                        n += 1
                        nop.engine = inst.engine
                        nop.sync_info = mybir.SyncInfo(on_wait=[w], on_update=[])
                        nc.register_instruction(nop, overwrite=True)
                        new_il.append(nop)
                    inst.sync_info = mybir.SyncInfo(
                        on_wait=[waits[-1]], on_update=list(si.on_update)
                    )
                new_il.append(inst)
            bb.instructions = new_il
    return n


def _build_nc():
    import concourse.bass as bass
    import concourse.mybir as mybir
    import concourse.tile as tile

    f32 = mybir.dt.float32
    bf = mybir.dt.bfloat16
    fp16 = mybir.dt.float16

    nc = bass.Bass()
    c5_d = nc.dram_tensor("c5", [96, B * NST * 128], bf, kind="ExternalInput")
    p5_d = nc.dram_tensor("p5", [96, N_JOBS * JM], bf, kind="ExternalInput")
    ft_d = nc.dram_tensor("ft", [128, B * NT * F], fp16, kind="ExternalInput")
    wr_d = nc.dram_tensor("wr", [128, M * OUT_D], bf, kind="ExternalInput")
    zz_d = nc.dram_tensor("zz", [1, 128 + JM], bf, kind="ExternalInput")
    out_d = nc.dram_tensor("out", [N_JOBS * L_SLAB, OUT_D], f32, kind="ExternalOutput")

    with (
        nc.allow_low_precision(reason="split-bf16 matmul is ~24-bit exact"),
        tile.TileContext(nc) as tc,
    ):
        with (
            tc.tile_pool(name="sq", bufs=2, space="PSUM") as sqpool,
            tc.tile_pool(name="acc", bufs=2, space="PSUM") as accpool,
            tc.tile_pool(name="const", bufs=1) as cpool,
            tc.tile_pool(name="kt", bufs=3) as ktpool,
            tc.tile_pool(name="sb", bufs=1) as sbpool,
        ):
            # ACT reciprocal table preload on the (tiny, first-DMA'd) zero
            # tile — the implied ACT_TABLE_LOAD overlaps the input DMAs
            zzs = cpool.tile([1, 128 + JM], bf)
            nc.sync.dma_start(zzs[:], zz_d[:, :])
            dumout = cpool.tile([1, 2], fp16)
            _act_reciprocal(nc, dumout[:], zzs[0:1, 0:2])

            # input DMAs, split so early compute can start ASAP
            c5s = cpool.tile([96, B * NST * 128], bf)
            nc.sync.dma_start(c5s[:, 0:128], c5_d[:, 0:128])
            nc.sync.dma_start(c5s[:, 128:512], c5_d[:, 128:512])
            nc.sync.dma_start(c5s[:, 512:1408], c5_d[:, 512:1408])
            nc.sync.dma_start(c5s[:, 1408:2816], c5_d[:, 1408:2816])
            p5s = cpool.tile([96, N_JOBS * JM], bf)
            nc.sync.dma_start(p5s[:, 0:JM], p5_d[:, 0:JM])
            nc.sync.dma_start(p5s[:, JM:], p5_d[:, JM:])
            fts = cpool.tile([128, B * NT * F], fp16)
            nc.sync.dma_start(fts[:, 0:512], ft_d[:, 0:512])
            nc.sync.dma_start(fts[:, 512:1024], ft_d[:, 512:1024])
            wrs = cpool.tile([128, M * OUT_D], bf)
            nc.sync.dma_start(wrs[:], wr_d[:, :])

            aggS = sbpool.tile([128, M * N_JOBS * L_SLAB], bf)
            aggS_v = aggS[:].rearrange("p (m jl) -> p m jl", m=M)

            for jj in range(N_JOBS):
                b = jj // N_SLABS
                agg = accpool.tile([128, JM], f32, tag="agg")
                # open the accumulation: clears has_written for the bank,
                # writes zeros over the full [128, JM] region
                nc.tensor.matmul(
                    agg[:, :],
                    lhsT=zzs[0:1, 0:128],
                    rhs=zzs[0:1, 128 : 128 + JM],
                    start=True,
                    stop=False,
                    skip_group_check=True,
                )
                for s in range(NST):
                    cn = 3 if s < NST - 1 else NT - 3 * (NST - 1)
                    sq = sqpool.tile([128, 3 * 512], f32, tag="sq")
                    for j in range(cn):
                        nc.tensor.matmul(
                            sq[:, 512 * j : 512 * j + JM],
                            lhsT=c5s[32 * j : 32 * j + K5, (b * NST + s) * 128 : (b * NST + s + 1) * 128],
                            rhs=p5s[32 * j : 32 * j + K5, jj * JM : (jj + 1) * JM],
                            start=True,
                            stop=True,
                        )
                    kt = ktpool.tile([128, 3 * JM], fp16, tag="kt")
                    sq_ap = sq[:].rearrange("p (c x) -> p c x", c=3)
                    kt_ap = kt[:].rearrange("p (c x) -> p c x", c=3, x=JM)
                    # reciprocal split by columns: ACT (1.04 ns/elem) takes
                    # [0:SPL], DVE's iterative divide (~6.4 ns/elem) takes
                    # [SPL:JM]; both read the same PSUM banks concurrently
                    _act_reciprocal(
                        nc, kt_ap[:, 0:cn, 0:SPL], sq_ap[:, 0:cn, 0:SPL]
                    )
                    nc.vector.reciprocal(
                        kt_ap[:, 0:cn, SPL:JM], sq_ap[:, 0:cn, SPL:JM]
                    )
                    for j in range(cn):
                        t = 3 * s + j
                        g = t % 4
                        nc.tensor.matmul(
                            agg[32 * g : 32 * g + 16, :],
                            lhsT=fts[:, (b * NT + t) * F : (b * NT + t + 1) * F],
                            rhs=kt[:, JM * j : JM * (j + 1)],
                            start=False,
                            stop=(t == NT - 1),
                            tile_position=(0, 32 * g),
                            skip_group_check=True,
                        )
                # agg psum -> SBUF bf16, laid out [128, (m, jl)] for the weighter
                nc.vector.tensor_copy(
                    aggS_v[:, :, jj * L_SLAB : (jj + 1) * L_SLAB],
                    agg[:].rearrange("p (m l) -> p m l", m=M),
                )

            # weighter: single bf16 pass; K=(g,f)=128 folds the col-group sum
            JL = N_JOBS * L_SLAB
            op = accpool.tile([128, OUT_D], f32, tag="agg")
            for mi in range(M):
                nc.tensor.matmul(
                    op[0:JL, :],
                    lhsT=aggS[:, mi * JL : (mi + 1) * JL],
                    rhs=wrs[:, mi * OUT_D : (mi + 1) * OUT_D],
                    start=(mi == 0),
                    stop=(mi == M - 1),
                )
            oS = sbpool.tile([JL, OUT_D], f32)
            nc.vector.tensor_copy(oS[:], op[0:JL, :])
            nc.sync.dma_start(out_d[:, :], oS[:])

    _split_multi_waits(nc)
    return nc


def _get_nc():
    global _NC
    if _NC is None:
        _NC = _build_nc()
    return _NC


def _prep_shared(points, W_weighter):
    coords = points[:, :, :D].astype(np.float64)           # [B, n, 3]
    feats = points[:, :, D:].astype(np.float32)            # [B, n, f]
    q = 10.0 * (coords**2).sum(-1)                         # [B, n] f64

    # c5 rows (bf16): per coordinate k the six cross rows pair as
    #   [c0, c0, c1, c1, c2, c0] x [p0, p1, p0, p1, p0, p2]
    # then [1,1,1] x [r0,r1,r2] and [q0,q1,q2] x [1,1,1].
    c5 = np.zeros((K5, B * N), bf16)
    for b in range(B):
        s = slice(b * N, (b + 1) * N)
        for k in range(D):
            c0, c1, c2 = _split3_bf16(coords[b, :, k])
            base = 6 * k
            c5[base + 0, s] = c0
            c5[base + 1, s] = c0
            c5[base + 2, s] = c1
            c5[base + 3, s] = c1
            c5[base + 4, s] = c2
            c5[base + 5, s] = c0
        c5[18:21, s] = 1.0
        q0, q1, q2 = _split3_bf16(q[b])
        c5[21, s] = q0
        c5[22, s] = q1
        c5[23, s] = q2

    # c5r: chunk t=3s+j of batch b -> rows 32j..32j+23, col block (b*NST+s)
    c5v = c5.reshape(K5, B, NT, 128)
    c5r = np.zeros((96, B * NST * 128), bf16)
    for b in range(B):
        for t in range(NT):
            s_, j = t // 3, t % 3
            c5r[32 * j : 32 * j + K5, (b * NST + s_) * 128 : (b * NST + s_ + 1) * 128] = c5v[:, b, t]

    # ft[p, (b, t, f)] = feats[b, t*128+p, f]   (fp16)
    ft = (
        np.ascontiguousarray(feats.reshape(B, NT, 128, F).transpose(2, 0, 1, 3))
        .reshape(128, B * NT * F)
        .astype(np.float16)
    )

    # wr[32g + f, (m, o)] = W[(m*F+f), o] * (8/n) bf16; rows 32g+16..31 = 0.
    # (u is scaled by 8 on the probe side so fp16 kern=1/(8u') stays normal.)
    w8 = np.ascontiguousarray(
        (W_weighter.astype(np.float64) * (8.0 / N)).reshape(M, F, OUT_D).transpose(1, 0, 2)
    ).reshape(F, M * OUT_D).astype(bf16)
    wr = np.zeros((128, M * OUT_D), bf16)
    for g in range(4):
        wr[32 * g : 32 * g + F, :] = w8
    return c5r, ft, wr


def _prep_probes5(centers, core):
    cen = centers[:, core * L_LOC : (core + 1) * L_LOC, :].astype(np.float64)  # [B, 32, 3]
    p5 = np.zeros((K5, N_JOBS * JM), bf16)
    for b in range(B):
        for sl_i in range(N_SLABS):
            jj = b * N_SLABS + sl_i
            s = slice(jj * JM, (jj + 1) * JM)
            sl = cen[b, sl_i * L_SLAB : (sl_i + 1) * L_SLAB]       # [16, 3]
            pf = sl[:, None, :] + PROBES[None].astype(np.float64)  # [16, 26, 3]
            mlf = pf.transpose(1, 0, 2).reshape(JM, 3)             # (m, l') major
            for k in range(D):
                p0, p1, p2 = _split3_bf16(8.0 * -20.0 * mlf[:, k])
                base = 6 * k
                p5[base + 0, s] = p0
                p5[base + 1, s] = p1
                p5[base + 2, s] = p0
                p5[base + 3, s] = p1
                p5[base + 4, s] = p0
                p5[base + 5, s] = p2
            r = 8.0 * (10.0 * (mlf**2).sum(-1) + 1.0)              # [JM] f64
            r0, r1, r2 = _split3_bf16(r)
            p5[18, s] = r0
            p5[19, s] = r1
            p5[20, s] = r2
            p5[21:24, s] = 8.0
    # replicate to row groups 0/32/64 for row-tiled matmul1
    p5r = np.zeros((96, N_JOBS * JM), bf16)
    for g in range(3):
        p5r[32 * g : 32 * g + K5, :] = p5
    return p5r


def _in_maps(points, centers, W_weighter):
    c5r, ft, wr = _prep_shared(points, W_weighter)
    zz = np.zeros((1, 128 + JM), bf16)
    return [
        {"c5": c5r, "ft": ft, "p5": _prep_probes5(centers, core), "wr": wr, "zz": zz}
        for core in range(N_CORES)
    ]


def kernel(points, centers, W_weighter, b_weighter):
    from concourse.bass_utils import run_bass_kernel_spmd

    points = np.asarray(points)
    centers = np.asarray(centers)
    W_weighter = np.asarray(W_weighter)
    b_weighter = np.asarray(b_weighter)

    nc = _get_nc()
    in_maps = _in_maps(points, centers, W_weighter)
    res = run_bass_kernel_spmd(nc, in_maps, core_ids=list(range(N_CORES))).results

    out = np.empty((B, L, OUT_D), np.float32)
    for core in range(N_CORES):
        r = res[core]["out"]  # [(jj, l'), OUT_D]
        for jj in range(N_JOBS):
            b, s = jj // N_SLABS, jj % N_SLABS
            lo = core * L_LOC + s * L_SLAB
            out[b, lo : lo + L_SLAB] = r[jj * L_SLAB : (jj + 1) * L_SLAB]
    out += b_weighter.astype(np.float32)[None, None, :]
    return out


# revision 10
# speedup vs baseline: 1.0048x; 1.0048x over previous
"""Trainium2 Bass kernel for nn_CrossConvLayerV2 (gnn_message_passing).

Math (reference):
    coords = points[..., :3]; feats = points[..., 3:]          # [B,n,3], [B,n,f]
    probes[b,l,m] = centers[b,l] + PROBES[m]                    # [B,l,m,3]
    sq[b,l,m,n]  = ||coords[b,n] - probes[b,l,m]||^2
    kern         = C / (sq + C)          (C = 0.1)
    agg[b,l,m,f] = (1/n) sum_n kern * feats
    out[b,l,:]   = agg.reshape(l, m*f) @ W + bias               # [B,l,256]

Strategy (v2):
  - Shard centers dim l (256) over 8 cores -> 32 centers/core, zero
    communication; the host gathers the 8 [B,32,256] shards.
  - Per job (b, 16-center slab) and per 128-point chunk t:
      u = 8*(10*sq+1) via ONE K=24 bf16 matmul of "expanded" vectors
        (3x bf16 splits -> ~24-bit exact; see _prep helpers).
      kern = 1/u on the ACT engine (Reciprocal LUT), fp16.
      agg[(g,f), (m,l')] += feats_chunk^T @ kern^T  (PSUM accumulate)
      out = sum_m aggS_m^T @ W_rep_m  (bf16, single pass)
  - v2 performance structure (vs v1, 151.9us -> target ~65us):
      * 3-chunk PSUM supertiles (3 banks): ONE ACT reciprocal instruction
        per 3 chunks (FD=1248) amortizes the ~310-cycle ACT fixed cost.
      * matmul1 row-tiled: chunks t%3 -> PE row groups 0/32/64, three
        concurrent K=24 matmuls (c5/p5 replicated per row group).
      * matmul2 col-tiled: chunks t%4 -> PE col groups, M=16 outputs land
        on agg partitions 32g..32g+15; a K=1 zero-matmul opens the
        accumulation (clears has_written for the whole bank ONCE).
      * group-sum over g is folded into the weighter: W replicated at
        partition rows 32g+f (rows 32g+16..31 zero).
      * weighter is a single bf16 pass (error budget ~0.4% << 2e-2).
  - This walrus build encodes at most ONE semaphore wait per instruction;
    a post-build pass splits multi-wait instructions into single-wait
    NoOp carriers.
"""

import sys

sys.path.insert(0, "/opt/trn_rl_repo")

import numpy as np
import ml_dtypes

# ---- problem constants (hardcoded per contract) ----
B, N, L, D, F = 2, 4096, 256, 3, 16
M = 26
OUT_D = 256
COEFF = 0.1
DIST = 3.0
N_CORES = 8
L_LOC = L // N_CORES          # 32 centers per core
N_SLABS = 2                   # jobs per batch elem per core
L_SLAB = L_LOC // N_SLABS     # 16 centers per job
JM = M * L_SLAB               # 416 = free dim of kern^T tiles
N_JOBS = B * N_SLABS          # 4 jobs per core
NT = N // 128                 # 32 n-chunks
K5 = 24                       # expanded-distance contraction depth
NST = (NT + 2) // 3           # 11 supertiles per job (10x3 + 1x2)
SPL = 364                     # reciprocal column split: ACT [0:SPL], DVE rest
bf16 = ml_dtypes.bfloat16


def _make_probes() -> np.ndarray:
    angles = np.array(
        [[j * 0.125 - 0.125, i * 0.125 + (j - 1) * 0.0625] for j in range(3) for i in range(8)]
        + [[-0.25, 0.0], [0.25, 0.0]],
        dtype=np.float64,
    ) * (2.0 * np.pi)
    a, b = angles[:, 0], angles[:, 1]
    pts = np.stack([np.sin(a), np.cos(a) * np.cos(b), np.cos(a) * np.sin(b)], axis=-1) * DIST
    return pts.astype(np.float32)  # [26, 3]


PROBES = _make_probes()


def _split3_bf16(x):
    """x (f64) -> three bf16 arrays whose sum approximates x to ~24 bits."""
    x0 = x.astype(bf16)
    r1 = x - x0.astype(np.float64)
    x1 = r1.astype(bf16)
    x2 = (r1 - x1.astype(np.float64)).astype(bf16)
    return x0, x1, x2


_NC = None


def _act_reciprocal(nc, out_ap, in_ap):
    """nc.scalar.activation(func=Reciprocal) minus the library guard.
    out = 1/in_ on the ACT engine (LUT path; measured ~1.2e-5 rel here)."""
    import concourse.mybir as mybir

    eng = nc.scalar
    inputs = [eng.lower_ap(in_ap)]
    for val in (0.0, 1.0, 0.0):  # bias, scale, alpha — immediates
        inputs.append(mybir.ImmediateValue(dtype=mybir.dt.float32, value=val))
    return eng.add_instruction(
        mybir.InstActivation(
            name=nc.get_next_instruction_name(),
            func=mybir.ActivationFunctionType.Reciprocal,
            ins=inputs,
            outs=[eng.lower_ap(out_ap)],
        )
    )


def _split_multi_waits(nc):
    """This walrus build encodes at most ONE semaphore wait per instruction.
    Split every instruction with k>1 waits into (k-1) single-wait NoOps on
    the same engine immediately before it — identical blocking semantics."""
    import concourse.mybir as mybir

    n = 0
    for f in nc.m.functions:
        for bb in f.blocks:
            new_il = []
            for inst in bb.instructions:
                si = inst.sync_info
                waits = list(si.on_wait) if si is not None else []
                if len(waits) > 1:
                    for w in waits[:-1]:
                        nop = mybir.InstNoOp(name=f"{inst.name}-wsplit{n}", ins=[], outs=[])
                        n += 1
                        nop.engine = inst.engine
                        nop.sync_info = mybir.SyncInfo(on_wait=[w], on_update=[])
                        nc.register_instruction(nop, overwrite=True)
                        new_il.append(nop)
                    inst.sync_info = mybir.SyncInfo(
                        on_wait=[waits[-1]], on_update=list(si.on_update)
                    )
                new_il.append(inst)
            bb.instructions = new_il
    return n


def _build_nc():
    import concourse.bass as bass
    import concourse.mybir as mybir
    import concourse.tile as tile

    f32 = mybir.dt.float32
    bf = mybir.dt.bfloat16
    fp16 = mybir.dt.float16

    nc = bass.Bass()
    c5_d = nc.dram_tensor("c5", [96, B * NST * 128], bf, kind="ExternalInput")
    p5_d = nc.dram_tensor("p5", [96, N_JOBS * JM], bf, kind="ExternalInput")
    ft_d = nc.dram_tensor("ft", [128, B * NT * F], fp16, kind="ExternalInput")
    wr_d = nc.dram_tensor("wr", [128, M * OUT_D], bf, kind="ExternalInput")
    zz_d = nc.dram_tensor("zz", [1, 128 + JM], bf, kind="ExternalInput")
    out_d = nc.dram_tensor("out", [N_JOBS * L_SLAB, OUT_D], f32, kind="ExternalOutput")

    with (
        nc.allow_low_precision(reason="split-bf16 matmul is ~24-bit exact"),
        tile.TileContext(nc) as tc,
    ):
        with (
            tc.tile_pool(name="sq", bufs=2, space="PSUM") as sqpool,
            tc.tile_pool(name="acc", bufs=2, space="PSUM") as accpool,
            tc.tile_pool(name="const", bufs=1) as cpool,
            tc.tile_pool(name="kt", bufs=3) as ktpool,
            tc.tile_pool(name="sb", bufs=1) as sbpool,
        ):
            # ACT reciprocal table preload on the (tiny, first-DMA'd) zero
            # tile — the implied ACT_TABLE_LOAD overlaps the input DMAs
            zzs = cpool.tile([1, 128 + JM], bf)
            nc.sync.dma_start(zzs[:], zz_d[:, :])
            dumout = cpool.tile([1, 2], fp16)
            _act_reciprocal(nc, dumout[:], zzs[0:1, 0:2])

            # input DMAs, split so early compute can start ASAP
            c5s = cpool.tile([96, B * NST * 128], bf)
            nc.sync.dma_start(c5s[:, 0:128], c5_d[:, 0:128])
            nc.sync.dma_start(c5s[:, 128:512], c5_d[:, 128:512])
            nc.sync.dma_start(c5s[:, 512:1408], c5_d[:, 512:1408])
            nc.sync.dma_start(c5s[:, 1408:2816], c5_d[:, 1408:2816])
            p5s = cpool.tile([96, N_JOBS * JM], bf)
            nc.sync.dma_start(p5s[:, 0:JM], p5_d[:, 0:JM])
            nc.sync.dma_start(p5s[:, JM:], p5_d[:, JM:])
            fts = cpool.tile([128, B * NT * F], fp16)
            nc.sync.dma_start(fts[:, 0:512], ft_d[:, 0:512])
            nc.sync.dma_start(fts[:, 512:1024], ft_d[:, 512:1024])
            wrs = cpool.tile([128, M * OUT_D], bf)
            nc.sync.dma_start(wrs[:], wr_d[:, :])

            aggS = sbpool.tile([128, M * N_JOBS * L_SLAB], bf)
            aggS_v = aggS[:].rearrange("p (m jl) -> p m jl", m=M)

            # Flat software pipeline over all (job, supertile) steps: issue
            # mm1(step k+1) + reciprocals(k+1) BEFORE mm2(step k). The PE
            # engine queue is strict FIFO, so this keeps mm1 ahead of the
            # kt-gated mm2 in the queue — the elementwise engines never
            # wait for a full PE round-trip between supertiles.
            steps = [(jj, s) for jj in range(N_JOBS) for s in range(NST)]
            aggs = {}
            kts = {}
            for idx in range(len(steps) + 1):
                if idx < len(steps):
                    jj, s = steps[idx]
                    b = jj // N_SLABS
                    if s == 0:
                        agg = accpool.tile([128, JM], f32, tag="agg", name=f"agg{jj}")
                        aggs[jj] = agg
                        # open the accumulation: clears has_written for the
                        # bank, writes zeros over the full [128, JM] region
                        nc.tensor.matmul(
                            agg[:, :],
                            lhsT=zzs[0:1, 0:128],
                            rhs=zzs[0:1, 128 : 128 + JM],
                            start=True,
                            stop=False,
                            skip_group_check=True,
                        )
                    cn = 3 if s < NST - 1 else NT - 3 * (NST - 1)
                    sq = sqpool.tile([128, 3 * 512], f32, tag="sq")
                    for j in range(cn):
                        nc.tensor.matmul(
                            sq[:, 512 * j : 512 * j + JM],
                            lhsT=c5s[32 * j : 32 * j + K5, (b * NST + s) * 128 : (b * NST + s + 1) * 128],
                            rhs=p5s[32 * j : 32 * j + K5, jj * JM : (jj + 1) * JM],
                            start=True,
                            stop=True,
                        )
                    kt = ktpool.tile([128, 3 * JM], fp16, tag="kt")
                    kts[(jj, s)] = kt
                    sq_ap = sq[:].rearrange("p (c x) -> p c x", c=3)
                    kt_ap = kt[:].rearrange("p (c x) -> p c x", c=3, x=JM)
                    # reciprocal split by columns: ACT (1.04 ns/elem) takes
                    # [0:SPL], DVE's iterative divide (~6.4 ns/elem) takes
                    # [SPL:JM]; both read the same PSUM banks concurrently
                    _act_reciprocal(
                        nc, kt_ap[:, 0:cn, 0:SPL], sq_ap[:, 0:cn, 0:SPL]
                    )
                    nc.vector.reciprocal(
                        kt_ap[:, 0:cn, SPL:JM], sq_ap[:, 0:cn, SPL:JM]
                    )
                if idx >= 1:
                    jj, s = steps[idx - 1]
                    b = jj // N_SLABS
                    cn = 3 if s < NST - 1 else NT - 3 * (NST - 1)
                    kt = kts.pop((jj, s))
                    agg = aggs[jj]
                    for j in range(cn):
                        t = 3 * s + j
                        g = t % 4
                        nc.tensor.matmul(
                            agg[32 * g : 32 * g + 16, :],
                            lhsT=fts[:, (b * NT + t) * F : (b * NT + t + 1) * F],
                            rhs=kt[:, JM * j : JM * (j + 1)],
                            start=False,
                            stop=(t == NT - 1),
                            tile_position=(0, 32 * g),
                            skip_group_check=True,
                        )
                    if s == NST - 1:
                        # agg psum -> SBUF bf16, [128, (m, jl)] for the weighter
                        nc.vector.tensor_copy(
                            aggS_v[:, :, jj * L_SLAB : (jj + 1) * L_SLAB],
                            agg[:].rearrange("p (m l) -> p m l", m=M),
                        )

            # weighter: single bf16 pass; K=(g,f)=128 folds the col-group sum
            JL = N_JOBS * L_SLAB
            op = accpool.tile([128, OUT_D], f32, tag="agg")
            for mi in range(M):
                nc.tensor.matmul(
                    op[0:JL, :],
                    lhsT=aggS[:, mi * JL : (mi + 1) * JL],
                    rhs=wrs[:, mi * OUT_D : (mi + 1) * OUT_D],
                    start=(mi == 0),
                    stop=(mi == M - 1),
                )
            oS = sbpool.tile([JL, OUT_D], f32)
            nc.vector.tensor_copy(oS[:], op[0:JL, :])
            nc.sync.dma_start(out_d[:, :], oS[:])

    _split_multi_waits(nc)
    return nc


def _get_nc():
    global _NC
    if _NC is None:
        _NC = _build_nc()
    return _NC


def _prep_shared(points, W_weighter):
    coords = points[:, :, :D].astype(np.float64)           # [B, n, 3]
    feats = points[:, :, D:].astype(np.float32)            # [B, n, f]
    q = 10.0 * (coords**2).sum(-1)                         # [B, n] f64

    # c5 rows (bf16): per coordinate k the six cross rows pair as
    #   [c0, c0, c1, c1, c2, c0] x [p0, p1, p0, p1, p0, p2]
    # then [1,1,1] x [r0,r1,r2] and [q0,q1,q2] x [1,1,1].
    c5 = np.zeros((K5, B * N), bf16)
    for b in range(B):
        s = slice(b * N, (b + 1) * N)
        for k in range(D):
            c0, c1, c2 = _split3_bf16(coords[b, :, k])
            base = 6 * k
            c5[base + 0, s] = c0
            c5[base + 1, s] = c0
            c5[base + 2, s] = c1
            c5[base + 3, s] = c1
            c5[base + 4, s] = c2
            c5[base + 5, s] = c0
        c5[18:21, s] = 1.0
        q0, q1, q2 = _split3_bf16(q[b])
        c5[21, s] = q0
        c5[22, s] = q1
        c5[23, s] = q2

    # c5r: chunk t=3s+j of batch b -> rows 32j..32j+23, col block (b*NST+s)
    c5v = c5.reshape(K5, B, NT, 128)
    c5r = np.zeros((96, B * NST * 128), bf16)
    for b in range(B):
        for t in range(NT):
            s_, j = t // 3, t % 3
            c5r[32 * j : 32 * j + K5, (b * NST + s_) * 128 : (b * NST + s_ + 1) * 128] = c5v[:, b, t]

    # ft[p, (b, t, f)] = feats[b, t*128+p, f]   (fp16)
    ft = (
        np.ascontiguousarray(feats.reshape(B, NT, 128, F).transpose(2, 0, 1, 3))
        .reshape(128, B * NT * F)
        .astype(np.float16)
    )

    # wr[32g + f, (m, o)] = W[(m*F+f), o] * (8/n) bf16; rows 32g+16..31 = 0.
    # (u is scaled by 8 on the probe side so fp16 kern=1/(8u') stays normal.)
    w8 = np.ascontiguousarray(
        (W_weighter.astype(np.float64) * (8.0 / N)).reshape(M, F, OUT_D).transpose(1, 0, 2)
    ).reshape(F, M * OUT_D).astype(bf16)
    wr = np.zeros((128, M * OUT_D), bf16)
    for g in range(4):
        wr[32 * g : 32 * g + F, :] = w8
    return c5r, ft, wr


def _prep_probes5(centers, core):
    cen = centers[:, core * L_LOC : (core + 1) * L_LOC, :].astype(np.float64)  # [B, 32, 3]
    p5 = np.zeros((K5, N_JOBS * JM), bf16)
    for b in range(B):
        for sl_i in range(N_SLABS):
            jj = b * N_SLABS + sl_i
            s = slice(jj * JM, (jj + 1) * JM)
            sl = cen[b, sl_i * L_SLAB : (sl_i + 1) * L_SLAB]       # [16, 3]
            pf = sl[:, None, :] + PROBES[None].astype(np.float64)  # [16, 26, 3]
            mlf = pf.transpose(1, 0, 2).reshape(JM, 3)             # (m, l') major
            for k in range(D):
                p0, p1, p2 = _split3_bf16(8.0 * -20.0 * mlf[:, k])
                base = 6 * k
                p5[base + 0, s] = p0
                p5[base + 1, s] = p1
                p5[base + 2, s] = p0
                p5[base + 3, s] = p1
                p5[base + 4, s] = p0
                p5[base + 5, s] = p2
            r = 8.0 * (10.0 * (mlf**2).sum(-1) + 1.0)              # [JM] f64
            r0, r1, r2 = _split3_bf16(r)
            p5[18, s] = r0
            p5[19, s] = r1
            p5[20, s] = r2
            p5[21:24, s] = 8.0
    # replicate to row groups 0/32/64 for row-tiled matmul1
    p5r = np.zeros((96, N_JOBS * JM), bf16)
    for g in range(3):
        p5r[32 * g : 32 * g + K5, :] = p5
    return p5r


def _in_maps(points, centers, W_weighter):
    c5r, ft, wr = _prep_shared(points, W_weighter)
    zz = np.zeros((1, 128 + JM), bf16)
    return [
        {"c5": c5r, "ft": ft, "p5": _prep_probes5(centers, core), "wr": wr, "zz": zz}
        for core in range(N_CORES)
    ]


def kernel(points, centers, W_weighter, b_weighter):
    from concourse.bass_utils import run_bass_kernel_spmd

    points = np.asarray(points)
    centers = np.asarray(centers)
    W_weighter = np.asarray(W_weighter)
    b_weighter = np.asarray(b_weighter)

    nc = _get_nc()
    in_maps = _in_maps(points, centers, W_weighter)
    res = run_bass_kernel_spmd(nc, in_maps, core_ids=list(range(N_CORES))).results

    out = np.empty((B, L, OUT_D), np.float32)
    for core in range(N_CORES):
        r = res[core]["out"]  # [(jj, l'), OUT_D]
        for jj in range(N_JOBS):
            b, s = jj // N_SLABS, jj % N_SLABS
            lo = core * L_LOC + s * L_SLAB
            out[b, lo : lo + L_SLAB] = r[jj * L_SLAB : (jj + 1) * L_SLAB]
    out += b_weighter.astype(np.float32)[None, None, :]
    return out


# revision 14
# speedup vs baseline: 1.1561x; 1.1505x over previous
"""Trainium2 Bass kernel for nn_CrossConvLayerV2 (gnn_message_passing).

Math (reference):
    coords = points[..., :3]; feats = points[..., 3:]          # [B,n,3], [B,n,f]
    probes[b,l,m] = centers[b,l] + PROBES[m]                    # [B,l,m,3]
    sq[b,l,m,n]  = ||coords[b,n] - probes[b,l,m]||^2
    kern         = C / (sq + C)          (C = 0.1)
    agg[b,l,m,f] = (1/n) sum_n kern * feats
    out[b,l,:]   = agg.reshape(l, m*f) @ W + bias               # [B,l,256]

Strategy (v2):
  - Shard centers dim l (256) over 8 cores -> 32 centers/core, zero
    communication; the host gathers the 8 [B,32,256] shards.
  - Per job (b, 16-center slab) and per 128-point chunk t:
      u = 8*(10*sq+1) via ONE K=24 bf16 matmul of "expanded" vectors
        (3x bf16 splits -> ~24-bit exact; see _prep helpers).
      kern = 1/u on the ACT engine (Reciprocal LUT), fp16.
      agg[(g,f), (m,l')] += feats_chunk^T @ kern^T  (PSUM accumulate)
      out = sum_m aggS_m^T @ W_rep_m  (bf16, single pass)
  - v2 performance structure (vs v1, 151.9us -> target ~65us):
      * 3-chunk PSUM supertiles (3 banks): ONE ACT reciprocal instruction
        per 3 chunks (FD=1248) amortizes the ~310-cycle ACT fixed cost.
      * matmul1 row-tiled: chunks t%3 -> PE row groups 0/32/64, three
        concurrent K=24 matmuls (c5/p5 replicated per row group).
      * matmul2 col-tiled: chunks t%4 -> PE col groups, M=16 outputs land
        on agg partitions 32g..32g+15; a K=1 zero-matmul opens the
        accumulation (clears has_written for the whole bank ONCE).
      * group-sum over g is folded into the weighter: W replicated at
        partition rows 32g+f (rows 32g+16..31 zero).
      * weighter is a single bf16 pass (error budget ~0.4% << 2e-2).
  - This walrus build encodes at most ONE semaphore wait per instruction;
    a post-build pass splits multi-wait instructions into single-wait
    NoOp carriers.
"""

import sys

sys.path.insert(0, "/opt/trn_rl_repo")

import numpy as np
import ml_dtypes

# ---- problem constants (hardcoded per contract) ----
B, N, L, D, F = 2, 4096, 256, 3, 16
M = 26
OUT_D = 256
COEFF = 0.1
DIST = 3.0
N_CORES = 8
L_LOC = L // N_CORES          # 32 centers per core
N_SLABS = 2                   # jobs per batch elem per core
L_SLAB = L_LOC // N_SLABS     # 16 centers per job
JM = M * L_SLAB               # 416 = free dim of kern^T tiles
N_JOBS = B * N_SLABS          # 4 jobs per core
NT = N // 128                 # 32 n-chunks
K5 = 24                       # expanded-distance contraction depth
NST = (NT + 2) // 3           # 11 supertiles per job (10x3 + 1x2)
SPL = 364                     # reciprocal column split: ACT [0:SPL], DVE rest
bf16 = ml_dtypes.bfloat16


def _make_probes() -> np.ndarray:
    angles = np.array(
        [[j * 0.125 - 0.125, i * 0.125 + (j - 1) * 0.0625] for j in range(3) for i in range(8)]
        + [[-0.25, 0.0], [0.25, 0.0]],
        dtype=np.float64,
    ) * (2.0 * np.pi)
    a, b = angles[:, 0], angles[:, 1]
    pts = np.stack([np.sin(a), np.cos(a) * np.cos(b), np.cos(a) * np.sin(b)], axis=-1) * DIST
    return pts.astype(np.float32)  # [26, 3]


PROBES = _make_probes()


def _split3_bf16(x):
    """x (f64) -> three bf16 arrays whose sum approximates x to ~24 bits."""
    x0 = x.astype(bf16)
    r1 = x - x0.astype(np.float64)
    x1 = r1.astype(bf16)
    x2 = (r1 - x1.astype(np.float64)).astype(bf16)
    return x0, x1, x2


_NC = None


def _act_reciprocal(nc, out_ap, in_ap):
    """nc.scalar.activation(func=Reciprocal) minus the library guard.
    out = 1/in_ on the ACT engine (LUT path; measured ~1.2e-5 rel here)."""
    import concourse.mybir as mybir

    eng = nc.scalar
    inputs = [eng.lower_ap(in_ap)]
    for val in (0.0, 1.0, 0.0):  # bias, scale, alpha — immediates
        inputs.append(mybir.ImmediateValue(dtype=mybir.dt.float32, value=val))
    return eng.add_instruction(
        mybir.InstActivation(
            name=nc.get_next_instruction_name(),
            func=mybir.ActivationFunctionType.Reciprocal,
            ins=inputs,
            outs=[eng.lower_ap(out_ap)],
        )
    )


def _split_multi_waits(nc):
    """This walrus build encodes at most ONE semaphore wait per instruction.
    Split every instruction with k>1 waits into (k-1) single-wait NoOps on
    the same engine immediately before it — identical blocking semantics."""
    import concourse.mybir as mybir

    n = 0
    for f in nc.m.functions:
        for bb in f.blocks:
            new_il = []
            for inst in bb.instructions:
                si = inst.sync_info
                waits = list(si.on_wait) if si is not None else []
                if len(waits) > 1:
                    for w in waits[:-1]:
                        nop = mybir.InstNoOp(name=f"{inst.name}-wsplit{n}", ins=[], outs=[])
                        n += 1
                        nop.engine = inst.engine
                        nop.sync_info = mybir.SyncInfo(on_wait=[w], on_update=[])
                        nc.register_instruction(nop, overwrite=True)
                        new_il.append(nop)
                    inst.sync_info = mybir.SyncInfo(
                        on_wait=[waits[-1]], on_update=list(si.on_update)
                    )
                new_il.append(inst)
            bb.instructions = new_il
    return n


def _build_nc():
    import concourse.bass as bass
    import concourse.mybir as mybir
    import concourse.tile as tile

    f32 = mybir.dt.float32
    bf = mybir.dt.bfloat16
    fp16 = mybir.dt.float16

    nc = bass.Bass()
    c5_d = nc.dram_tensor("c5", [96, B * NST * 128], bf, kind="ExternalInput")
    p5_d = nc.dram_tensor("p5", [96, N_JOBS * JM], bf, kind="ExternalInput")
    ft_d = nc.dram_tensor("ft", [128, B * NT * F], fp16, kind="ExternalInput")
    wr_d = nc.dram_tensor("wr", [128, M * OUT_D], bf, kind="ExternalInput")
    zz_d = nc.dram_tensor("zz", [1, 128 + JM], bf, kind="ExternalInput")
    out_d = nc.dram_tensor("out", [N_JOBS * L_SLAB, OUT_D], f32, kind="ExternalOutput")

    with (
        nc.allow_low_precision(reason="split-bf16 matmul is ~24-bit exact"),
        tile.TileContext(nc) as tc,
    ):
        with (
            tc.tile_pool(name="sq", bufs=2, space="PSUM") as sqpool,
            tc.tile_pool(name="acc", bufs=2, space="PSUM") as accpool,
            tc.tile_pool(name="const", bufs=1) as cpool,
            tc.tile_pool(name="kt", bufs=4) as ktpool,
            tc.tile_pool(name="sb", bufs=1) as sbpool,
        ):
            # ACT reciprocal table preload on the (tiny, first-DMA'd) zero
            # tile — the implied ACT_TABLE_LOAD overlaps the input DMAs
            zzs = cpool.tile([1, 128 + JM], bf)
            nc.sync.dma_start(zzs[:], zz_d[:, :])
            dumout = cpool.tile([1, 2], fp16)
            _act_reciprocal(nc, dumout[:], zzs[0:1, 0:2])

            # input DMAs, critical pieces first: supertile-0 c5, job-0 p5,
            # batch-0 feats; bulk and the (end-only) weighter matrix last
            c5s = cpool.tile([96, B * NST * 128], bf)
            p5s = cpool.tile([96, N_JOBS * JM], bf)
            fts = cpool.tile([128, B * NT * F], fp16)
            wrs = cpool.tile([128, M * OUT_D], bf)
            nc.sync.dma_start(c5s[:, 0:128], c5_d[:, 0:128])
            nc.sync.dma_start(p5s[:, 0:JM], p5_d[:, 0:JM])
            nc.sync.dma_start(c5s[:, 128:512], c5_d[:, 128:512])
            nc.sync.dma_start(fts[:, 0:512], ft_d[:, 0:512])
            nc.sync.dma_start(c5s[:, 512:1408], c5_d[:, 512:1408])
            nc.sync.dma_start(p5s[:, JM:], p5_d[:, JM:])
            nc.sync.dma_start(c5s[:, 1408:2816], c5_d[:, 1408:2816])
            nc.sync.dma_start(fts[:, 512:1024], ft_d[:, 512:1024])
            nc.sync.dma_start(wrs[:], wr_d[:, :])

            aggS = sbpool.tile([128, M * N_JOBS * L_SLAB], bf)
            aggS_v = aggS[:].rearrange("p (m jl) -> p m jl", m=M)

            # Flat software pipeline over all (job, supertile) steps, with
            # mm2 lagging LAG supertiles behind mm1. The PE engine queue is
            # strict FIFO, so a lag of 2 guarantees that by the time mm2(k)
            # reaches the head of the queue its kt (ACT+DVE reciprocals of
            # step k) finished long ago — mm1/mm2 never stall the queue and
            # the elementwise engines stay saturated.
            LAG = 2
            steps = [(jj, s) for jj in range(N_JOBS) for s in range(NST)]
            aggs = {}
            kts = {}
            for idx in range(len(steps) + LAG):
                if idx < len(steps):
                    jj, s = steps[idx]
                    b = jj // N_SLABS
                    if s == 0:
                        agg = accpool.tile([128, JM], f32, tag="agg", name=f"agg{jj}")
                        aggs[jj] = agg
                        # open the accumulation: clears has_written for the
                        # bank, writes zeros over the full [128, JM] region
                        nc.tensor.matmul(
                            agg[:, :],
                            lhsT=zzs[0:1, 0:128],
                            rhs=zzs[0:1, 128 : 128 + JM],
                            start=True,
                            stop=False,
                            skip_group_check=True,
                        )
                    cn = 3 if s < NST - 1 else NT - 3 * (NST - 1)
                    sq = sqpool.tile([128, 3 * 512], f32, tag="sq")
                    for j in range(cn):
                        nc.tensor.matmul(
                            sq[:, 512 * j : 512 * j + JM],
                            lhsT=c5s[32 * j : 32 * j + K5, (b * NST + s) * 128 : (b * NST + s + 1) * 128],
                            rhs=p5s[32 * j : 32 * j + K5, jj * JM : (jj + 1) * JM],
                            start=True,
                            stop=True,
                        )
                    kt = ktpool.tile([128, 3 * JM], fp16, tag="kt")
                    kts[(jj, s)] = kt
                    sq_ap = sq[:].rearrange("p (c x) -> p c x", c=3)
                    kt_ap = kt[:].rearrange("p (c x) -> p c x", c=3, x=JM)
                    # reciprocal split by columns: ACT (1.04 ns/elem) takes
                    # [0:SPL], DVE's iterative divide (~6.4 ns/elem) takes
                    # [SPL:JM]; both read the same PSUM banks concurrently
                    _act_reciprocal(
                        nc, kt_ap[:, 0:cn, 0:SPL], sq_ap[:, 0:cn, 0:SPL]
                    )
                    nc.vector.reciprocal(
                        kt_ap[:, 0:cn, SPL:JM], sq_ap[:, 0:cn, SPL:JM]
                    )
                if idx >= LAG:
                    jj, s = steps[idx - LAG]
                    b = jj // N_SLABS
                    cn = 3 if s < NST - 1 else NT - 3 * (NST - 1)
                    kt = kts.pop((jj, s))
                    agg = aggs[jj]
                    for j in range(cn):
                        t = 3 * s + j
                        g = t % 4
                        nc.tensor.matmul(
                            agg[32 * g : 32 * g + 16, :],
                            lhsT=fts[:, (b * NT + t) * F : (b * NT + t + 1) * F],
                            rhs=kt[:, JM * j : JM * (j + 1)],
                            start=False,
                            stop=(t == NT - 1),
                            tile_position=(0, 32 * g),
                            skip_group_check=True,
                        )
                    if s == NST - 1:
                        # agg psum -> SBUF bf16, [128, (m, jl)] for the weighter
                        nc.vector.tensor_copy(
                            aggS_v[:, :, jj * L_SLAB : (jj + 1) * L_SLAB],
                            agg[:].rearrange("p (m l) -> p m l", m=M),
                        )

            # weighter: single bf16 pass; K=(g,f)=128 folds the col-group sum
            JL = N_JOBS * L_SLAB
            op = accpool.tile([128, OUT_D], f32, tag="agg")
            for mi in range(M):
                nc.tensor.matmul(
                    op[0:JL, :],
                    lhsT=aggS[:, mi * JL : (mi + 1) * JL],
                    rhs=wrs[:, mi * OUT_D : (mi + 1) * OUT_D],
                    start=(mi == 0),
                    stop=(mi == M - 1),
                )
            oS = sbpool.tile([JL, OUT_D], f32)
            nc.vector.tensor_copy(oS[:], op[0:JL, :])
            nc.sync.dma_start(out_d[:, :], oS[:])

    _split_multi_waits(nc)
    return nc


def _get_nc():
    global _NC
    if _NC is None:
        _NC = _build_nc()
    return _NC


def _prep_shared(points, W_weighter):
    coords = points[:, :, :D].astype(np.float64)           # [B, n, 3]
    feats = points[:, :, D:].astype(np.float32)            # [B, n, f]
    q = 10.0 * (coords**2).sum(-1)                         # [B, n] f64

    # c5 rows (bf16): per coordinate k the six cross rows pair as
    #   [c0, c0, c1, c1, c2, c0] x [p0, p1, p0, p1, p0, p2]
    # then [1,1,1] x [r0,r1,r2] and [q0,q1,q2] x [1,1,1].
    c5 = np.zeros((K5, B * N), bf16)
    for b in range(B):
        s = slice(b * N, (b + 1) * N)
        for k in range(D):
            c0, c1, c2 = _split3_bf16(coords[b, :, k])
            base = 6 * k
            c5[base + 0, s] = c0
            c5[base + 1, s] = c0
            c5[base + 2, s] = c1
            c5[base + 3, s] = c1
            c5[base + 4, s] = c2
            c5[base + 5, s] = c0
        c5[18:21, s] = 1.0
        q0, q1, q2 = _split3_bf16(q[b])
        c5[21, s] = q0
        c5[22, s] = q1
        c5[23, s] = q2

    # c5r: chunk t=3s+j of batch b -> rows 32j..32j+23, col block (b*NST+s)
    c5v = c5.reshape(K5, B, NT, 128)
    c5r = np.zeros((96, B * NST * 128), bf16)
    for b in range(B):
        for t in range(NT):
            s_, j = t // 3, t % 3
            c5r[32 * j : 32 * j + K5, (b * NST + s_) * 128 : (b * NST + s_ + 1) * 128] = c5v[:, b, t]

    # ft[p, (b, t, f)] = feats[b, t*128+p, f]   (fp16)
    ft = (
        np.ascontiguousarray(feats.reshape(B, NT, 128, F).transpose(2, 0, 1, 3))
        .reshape(128, B * NT * F)
        .astype(np.float16)
    )

    # wr[32g + f, (m, o)] = W[(m*F+f), o] * (8/n) bf16; rows 32g+16..31 = 0.
    # (u is scaled by 8 on the probe side so fp16 kern=1/(8u') stays normal.)
    w8 = np.ascontiguousarray(
        (W_weighter.astype(np.float64) * (8.0 / N)).reshape(M, F, OUT_D).transpose(1, 0, 2)
    ).reshape(F, M * OUT_D).astype(bf16)
    wr = np.zeros((128, M * OUT_D), bf16)
    for g in range(4):
        wr[32 * g : 32 * g + F, :] = w8
    return c5r, ft, wr


def _prep_probes5(centers, core):
    cen = centers[:, core * L_LOC : (core + 1) * L_LOC, :].astype(np.float64)  # [B, 32, 3]
    p5 = np.zeros((K5, N_JOBS * JM), bf16)
    for b in range(B):
        for sl_i in range(N_SLABS):
            jj = b * N_SLABS + sl_i
            s = slice(jj * JM, (jj + 1) * JM)
            sl = cen[b, sl_i * L_SLAB : (sl_i + 1) * L_SLAB]       # [16, 3]
            pf = sl[:, None, :] + PROBES[None].astype(np.float64)  # [16, 26, 3]
            mlf = pf.transpose(1, 0, 2).reshape(JM, 3)             # (m, l') major
            for k in range(D):
                p0, p1, p2 = _split3_bf16(8.0 * -20.0 * mlf[:, k])
                base = 6 * k
                p5[base + 0, s] = p0
                p5[base + 1, s] = p1
                p5[base + 2, s] = p0
                p5[base + 3, s] = p1
                p5[base + 4, s] = p0
                p5[base + 5, s] = p2
            r = 8.0 * (10.0 * (mlf**2).sum(-1) + 1.0)              # [JM] f64
            r0, r1, r2 = _split3_bf16(r)
            p5[18, s] = r0
            p5[19, s] = r1
            p5[20, s] = r2
            p5[21:24, s] = 8.0
    # replicate to row groups 0/32/64 for row-tiled matmul1
    p5r = np.zeros((96, N_JOBS * JM), bf16)
    for g in range(3):
        p5r[32 * g : 32 * g + K5, :] = p5
    return p5r


def _in_maps(points, centers, W_weighter):
    c5r, ft, wr = _prep_shared(points, W_weighter)
    zz = np.zeros((1, 128 + JM), bf16)
    return [
        {"c5": c5r, "ft": ft, "p5": _prep_probes5(centers, core), "wr": wr, "zz": zz}
        for core in range(N_CORES)
    ]


def kernel(points, centers, W_weighter, b_weighter):
    from concourse.bass_utils import run_bass_kernel_spmd

    points = np.asarray(points)
    centers = np.asarray(centers)
    W_weighter = np.asarray(W_weighter)
    b_weighter = np.asarray(b_weighter)

    nc = _get_nc()
    in_maps = _in_maps(points, centers, W_weighter)
    res = run_bass_kernel_spmd(nc, in_maps, core_ids=list(range(N_CORES))).results

    out = np.empty((B, L, OUT_D), np.float32)
    for core in range(N_CORES):
        r = res[core]["out"]  # [(jj, l'), OUT_D]
        for jj in range(N_JOBS):
            b, s = jj // N_SLABS, jj % N_SLABS
            lo = core * L_LOC + s * L_SLAB
            out[b, lo : lo + L_SLAB] = r[jj * L_SLAB : (jj + 1) * L_SLAB]
    out += b_weighter.astype(np.float32)[None, None, :]
    return out


# revision 20
# speedup vs baseline: 1.2561x; 1.0865x over previous
"""Trainium2 Bass kernel for nn_CrossConvLayerV2 (gnn_message_passing).

Math (reference):
    coords = points[..., :3]; feats = points[..., 3:]          # [B,n,3], [B,n,f]
    probes[b,l,m] = centers[b,l] + PROBES[m]                    # [B,l,m,3]
    sq[b,l,m,n]  = ||coords[b,n] - probes[b,l,m]||^2
    kern         = C / (sq + C)          (C = 0.1)
    agg[b,l,m,f] = (1/n) sum_n kern * feats
    out[b,l,:]   = agg.reshape(l, m*f) @ W + bias               # [B,l,256]

Strategy (v2):
  - Shard centers dim l (256) over 8 cores -> 32 centers/core, zero
    communication; the host gathers the 8 [B,32,256] shards.
  - Per job (b, 16-center slab) and per 128-point chunk t:
      u = 8*(10*sq+1) via ONE K=24 bf16 matmul of "expanded" vectors
        (3x bf16 splits -> ~24-bit exact; see _prep helpers).
      kern = 1/u on the ACT engine (Reciprocal LUT), fp16.
      agg[(g,f), (m,l')] += feats_chunk^T @ kern^T  (PSUM accumulate)
      out = sum_m aggS_m^T @ W_rep_m  (bf16, single pass)
  - v2 performance structure (vs v1, 151.9us -> target ~65us):
      * 3-chunk PSUM supertiles (3 banks): ONE ACT reciprocal instruction
        per 3 chunks (FD=1248) amortizes the ~310-cycle ACT fixed cost.
      * matmul1 row-tiled: chunks t%3 -> PE row groups 0/32/64, three
        concurrent K=24 matmuls (c5/p5 replicated per row group).
      * matmul2 col-tiled: chunks t%4 -> PE col groups, M=16 outputs land
        on agg partitions 32g..32g+15; a K=1 zero-matmul opens the
        accumulation (clears has_written for the whole bank ONCE).
      * group-sum over g is folded into the weighter: W replicated at
        partition rows 32g+f (rows 32g+16..31 zero).
      * weighter is a single bf16 pass (error budget ~0.4% << 2e-2).
  - This walrus build encodes at most ONE semaphore wait per instruction;
    a post-build pass splits multi-wait instructions into single-wait
    NoOp carriers.
"""

import sys

sys.path.insert(0, "/opt/trn_rl_repo")

import numpy as np
import ml_dtypes

# ---- problem constants (hardcoded per contract) ----
B, N, L, D, F = 2, 4096, 256, 3, 16
M = 26
OUT_D = 256
COEFF = 0.1
DIST = 3.0
N_CORES = 8
L_LOC = L // N_CORES          # 32 centers per core
N_SLABS = 2                   # jobs per batch elem per core
L_SLAB = L_LOC // N_SLABS     # 16 centers per job
JM = M * L_SLAB               # 416 = free dim of kern^T tiles
N_JOBS = B * N_SLABS          # 4 jobs per core
NT = N // 128                 # 32 n-chunks
K5 = 24                       # expanded-distance contraction depth
NST = (NT + 2) // 3           # 11 supertiles per job (10x3 + 1x2)
SPL = 380                     # reciprocal column split: ACT [0:SPL], DVE rest
                              # (DVE share sized so its reciprocal always
                              # finishes before ACT -> sq-buffer release is
                              # ACT-driven with ~18% pipeline margin)
bf16 = ml_dtypes.bfloat16


def _make_probes() -> np.ndarray:
    angles = np.array(
        [[j * 0.125 - 0.125, i * 0.125 + (j - 1) * 0.0625] for j in range(3) for i in range(8)]
        + [[-0.25, 0.0], [0.25, 0.0]],
        dtype=np.float64,
    ) * (2.0 * np.pi)
    a, b = angles[:, 0], angles[:, 1]
    pts = np.stack([np.sin(a), np.cos(a) * np.cos(b), np.cos(a) * np.sin(b)], axis=-1) * DIST
    return pts.astype(np.float32)  # [26, 3]


PROBES = _make_probes()


def _split3_bf16(x):
    """x (f64) -> three bf16 arrays whose sum approximates x to ~24 bits."""
    x0 = x.astype(bf16)
    r1 = x - x0.astype(np.float64)
    x1 = r1.astype(bf16)
    x2 = (r1 - x1.astype(np.float64)).astype(bf16)
    return x0, x1, x2


_NC = None


def _act_reciprocal(nc, out_ap, in_ap):
    """nc.scalar.activation(func=Reciprocal) minus the library guard.
    out = 1/in_ on the ACT engine (LUT path; measured ~1.2e-5 rel here)."""
    import concourse.mybir as mybir

    eng = nc.scalar
    inputs = [eng.lower_ap(in_ap)]
    for val in (0.0, 1.0, 0.0):  # bias, scale, alpha — immediates
        inputs.append(mybir.ImmediateValue(dtype=mybir.dt.float32, value=val))
    return eng.add_instruction(
        mybir.InstActivation(
            name=nc.get_next_instruction_name(),
            func=mybir.ActivationFunctionType.Reciprocal,
            ins=inputs,
            outs=[eng.lower_ap(out_ap)],
        )
    )


def _split_multi_waits(nc):
    """This walrus build encodes at most ONE semaphore wait per instruction.
    Split every instruction with k>1 waits into (k-1) single-wait NoOps on
    the same engine immediately before it — identical blocking semantics."""
    import concourse.mybir as mybir

    n = 0
    for f in nc.m.functions:
        for bb in f.blocks:
            new_il = []
            for inst in bb.instructions:
                si = inst.sync_info
                waits = list(si.on_wait) if si is not None else []
                if len(waits) > 1:
                    for w in waits[:-1]:
                        nop = mybir.InstNoOp(name=f"{inst.name}-wsplit{n}", ins=[], outs=[])
                        n += 1
                        nop.engine = inst.engine
                        nop.sync_info = mybir.SyncInfo(on_wait=[w], on_update=[])
                        nc.register_instruction(nop, overwrite=True)
                        new_il.append(nop)
                    inst.sync_info = mybir.SyncInfo(
                        on_wait=[waits[-1]], on_update=list(si.on_update)
                    )
                new_il.append(inst)
            bb.instructions = new_il
    return n


def _build_nc():
    import concourse.bass as bass
    import concourse.mybir as mybir
    import concourse.tile as tile

    f32 = mybir.dt.float32
    bf = mybir.dt.bfloat16
    fp16 = mybir.dt.float16

    nc = bass.Bass()
    c5_d = nc.dram_tensor("c5", [96, B * NST * 128], bf, kind="ExternalInput")
    p5_d = nc.dram_tensor("p5", [96, N_JOBS * JM], bf, kind="ExternalInput")
    ft_d = nc.dram_tensor("ft", [128, B * NT * F], fp16, kind="ExternalInput")
    wr_d = nc.dram_tensor("wr", [128, M * OUT_D], bf, kind="ExternalInput")
    zz_d = nc.dram_tensor("zz", [1, 128 + JM], bf, kind="ExternalInput")
    # rows 0:64 hold the even-m weighter partial, rows 64:128 the odd-m
    # partial (PE col-group alternation); the host sums the halves
    out_d = nc.dram_tensor("out", [128, OUT_D], f32, kind="ExternalOutput")

    with (
        nc.allow_low_precision(reason="split-bf16 matmul is ~24-bit exact"),
        tile.TileContext(nc) as tc,
    ):
        with (
            tc.tile_pool(name="sq", bufs=2, space="PSUM") as sqpool,
            tc.tile_pool(name="acc", bufs=2, space="PSUM") as accpool,
            tc.tile_pool(name="const", bufs=1) as cpool,
            tc.tile_pool(name="kt", bufs=4) as ktpool,
            tc.tile_pool(name="sb", bufs=1) as sbpool,
        ):
            # ACT reciprocal table preload on the (tiny, first-DMA'd) zero
            # tile — the implied ACT_TABLE_LOAD overlaps the input DMAs
            zzs = cpool.tile([1, 128 + JM], bf)
            nc.sync.dma_start(zzs[:], zz_d[:, :])
            dumout = cpool.tile([1, 2], fp16)
            _act_reciprocal(nc, dumout[:], zzs[0:1, 0:2])

            # input DMAs, critical pieces first: supertile-0 c5, job-0 p5,
            # batch-0 feats; bulk and the (end-only) weighter matrix last
            c5s = cpool.tile([96, B * NST * 128], bf)
            p5s = cpool.tile([96, N_JOBS * JM], bf)
            fts = cpool.tile([128, B * NT * F], fp16)
            wrs = cpool.tile([128, M * OUT_D], bf)
            nc.sync.dma_start(c5s[:, 0:128], c5_d[:, 0:128])
            nc.sync.dma_start(p5s[:, 0:JM], p5_d[:, 0:JM])
            nc.sync.dma_start(c5s[:, 128:512], c5_d[:, 128:512])
            nc.sync.dma_start(fts[:, 0:512], ft_d[:, 0:512])
            nc.sync.dma_start(c5s[:, 512:1408], c5_d[:, 512:1408])
            nc.sync.dma_start(p5s[:, JM:], p5_d[:, JM:])
            nc.sync.dma_start(c5s[:, 1408:2816], c5_d[:, 1408:2816])
            nc.sync.dma_start(fts[:, 512:1024], ft_d[:, 512:1024])
            nc.sync.dma_start(wrs[:], wr_d[:, :])

            aggS = sbpool.tile([128, M * N_JOBS * L_SLAB], bf)
            aggS_v = aggS[:].rearrange("p (m jl) -> p m jl", m=M)

            # Flat software pipeline over all (job, supertile) steps, with
            # mm2 lagging LAG supertiles behind mm1. The PE engine queue is
            # strict FIFO, so a lag of 2 guarantees that by the time mm2(k)
            # reaches the head of the queue its kt (ACT+DVE reciprocals of
            # step k) finished long ago — mm1/mm2 never stall the queue and
            # the elementwise engines stay saturated.
            LAG = 2
            steps = [(jj, s) for jj in range(N_JOBS) for s in range(NST)]
            aggs = {}
            kts = {}
            for idx in range(len(steps) + LAG):
                if idx < len(steps):
                    jj, s = steps[idx]
                    b = jj // N_SLABS
                    if s == 0:
                        agg = accpool.tile([128, JM], f32, tag="agg", name=f"agg{jj}")
                        aggs[jj] = agg
                        # open the accumulation: clears has_written for the
                        # bank, writes zeros over the full [128, JM] region
                        nc.tensor.matmul(
                            agg[:, :],
                            lhsT=zzs[0:1, 0:128],
                            rhs=zzs[0:1, 128 : 128 + JM],
                            start=True,
                            stop=False,
                            skip_group_check=True,
                        )
                    cn = 3 if s < NST - 1 else NT - 3 * (NST - 1)
                    sq = sqpool.tile([128, 3 * 512], f32, tag="sq")
                    for j in range(cn):
                        nc.tensor.matmul(
                            sq[:, 512 * j : 512 * j + JM],
                            lhsT=c5s[32 * j : 32 * j + K5, (b * NST + s) * 128 : (b * NST + s + 1) * 128],
                            rhs=p5s[32 * j : 32 * j + K5, jj * JM : (jj + 1) * JM],
                            start=True,
                            stop=True,
                        )
                    kt = ktpool.tile([128, 3 * JM], fp16, tag="kt")
                    kts[(jj, s)] = kt
                    sq_ap = sq[:].rearrange("p (c x) -> p c x", c=3)
                    kt_ap = kt[:].rearrange("p (c x) -> p c x", c=3, x=JM)
                    # reciprocal split by columns: ACT (1.04 ns/elem) takes
                    # [0:SPL], DVE's iterative divide (~6.4 ns/elem) takes
                    # [SPL:JM]; both read the same PSUM banks concurrently
                    _act_reciprocal(
                        nc, kt_ap[:, 0:cn, 0:SPL], sq_ap[:, 0:cn, 0:SPL]
                    )
                    nc.vector.reciprocal(
                        kt_ap[:, 0:cn, SPL:JM], sq_ap[:, 0:cn, SPL:JM]
                    )
                if idx >= LAG:
                    jj, s = steps[idx - LAG]
                    b = jj // N_SLABS
                    cn = 3 if s < NST - 1 else NT - 3 * (NST - 1)
                    kt = kts.pop((jj, s))
                    agg = aggs[jj]
                    for j in range(cn):
                        t = 3 * s + j
                        g = t % 4
                        nc.tensor.matmul(
                            agg[32 * g : 32 * g + 16, :],
                            lhsT=fts[:, (b * NT + t) * F : (b * NT + t + 1) * F],
                            rhs=kt[:, JM * j : JM * (j + 1)],
                            start=False,
                            stop=(t == NT - 1),
                            tile_position=(0, 32 * g),
                            skip_group_check=True,
                        )
                    if s == NST - 1:
                        # agg psum -> SBUF bf16, [128, (m, jl)] for the weighter
                        nc.vector.tensor_copy(
                            aggS_v[:, :, jj * L_SLAB : (jj + 1) * L_SLAB],
                            agg[:].rearrange("p (m l) -> p m l", m=M),
                        )

            # weighter: single bf16 pass; K=(g,f)=128 folds the col-group sum.
            # Even m -> PE col-half 0 (out rows 0:64), odd m -> col-half 1
            # (out rows 64:128): consecutive LDWEIGHTS/MATMULs live in
            # different array halves, so they overlap instead of
            # serializing. mi=0's start=True clears the whole bank; every
            # later matmul (start=False) overwrites-then-accumulates via
            # has_written, including the first odd-m one.
            JL = N_JOBS * L_SLAB
            op = accpool.tile([128, OUT_D], f32, tag="agg")
            # full-height zero-matmul: clears has_written + zeroes all 128
            # partitions (a start=True on a 64-row AP only resets the rows
            # it writes)
            nc.tensor.matmul(
                op[:, :],
                lhsT=zzs[0:1, 0:128],
                rhs=zzs[0:1, 128 : 128 + OUT_D],
                start=True,
                stop=False,
                skip_group_check=True,
            )
            for mi in range(M):
                h = mi % 2
                nc.tensor.matmul(
                    op[JL * h : JL * (h + 1), :],
                    lhsT=aggS[:, mi * JL : (mi + 1) * JL],
                    rhs=wrs[:, mi * OUT_D : (mi + 1) * OUT_D],
                    start=False,
                    stop=(mi >= M - 2),
                    tile_position=(0, JL * h),
                    skip_group_check=True,
                )
            oS = sbpool.tile([128, OUT_D], f32)
            nc.vector.tensor_copy(oS[:], op[:, :])
            nc.sync.dma_start(out_d[:, :], oS[:, :])

    _split_multi_waits(nc)
    return nc


def _get_nc():
    global _NC
    if _NC is None:
        _NC = _build_nc()
    return _NC


def _prep_shared(points, W_weighter):
    coords = points[:, :, :D].astype(np.float64)           # [B, n, 3]
    feats = points[:, :, D:].astype(np.float32)            # [B, n, f]
    q = 10.0 * (coords**2).sum(-1)                         # [B, n] f64

    # c5 rows (bf16): per coordinate k the six cross rows pair as
    #   [c0, c0, c1, c1, c2, c0] x [p0, p1, p0, p1, p0, p2]
    # then [1,1,1] x [r0,r1,r2] and [q0,q1,q2] x [1,1,1].
    c5 = np.zeros((K5, B * N), bf16)
    for b in range(B):
        s = slice(b * N, (b + 1) * N)
        for k in range(D):
            c0, c1, c2 = _split3_bf16(coords[b, :, k])
            base = 6 * k
            c5[base + 0, s] = c0
            c5[base + 1, s] = c0
            c5[base + 2, s] = c1
            c5[base + 3, s] = c1
            c5[base + 4, s] = c2
            c5[base + 5, s] = c0
        c5[18:21, s] = 1.0
        q0, q1, q2 = _split3_bf16(q[b])
        c5[21, s] = q0
        c5[22, s] = q1
        c5[23, s] = q2

    # c5r: chunk t=3s+j of batch b -> rows 32j..32j+23, col block (b*NST+s)
    c5v = c5.reshape(K5, B, NT, 128)
    c5r = np.zeros((96, B * NST * 128), bf16)
    for b in range(B):
        for t in range(NT):
            s_, j = t // 3, t % 3
            c5r[32 * j : 32 * j + K5, (b * NST + s_) * 128 : (b * NST + s_ + 1) * 128] = c5v[:, b, t]

    # ft[p, (b, t, f)] = feats[b, t*128+p, f]   (fp16)
    ft = (
        np.ascontiguousarray(feats.reshape(B, NT, 128, F).transpose(2, 0, 1, 3))
        .reshape(128, B * NT * F)
        .astype(np.float16)
    )

    # wr[32g + f, (m, o)] = W[(m*F+f), o] * (8/n) bf16; rows 32g+16..31 = 0.
    # (u is scaled by 8 on the probe side so fp16 kern=1/(8u') stays normal.)
    w8 = np.ascontiguousarray(
        (W_weighter.astype(np.float64) * (8.0 / N)).reshape(M, F, OUT_D).transpose(1, 0, 2)
    ).reshape(F, M * OUT_D).astype(bf16)
    wr = np.zeros((128, M * OUT_D), bf16)
    for g in range(4):
        wr[32 * g : 32 * g + F, :] = w8
    return c5r, ft, wr


def _prep_probes5(centers, core):
    cen = centers[:, core * L_LOC : (core + 1) * L_LOC, :].astype(np.float64)  # [B, 32, 3]
    p5 = np.zeros((K5, N_JOBS * JM), bf16)
    for b in range(B):
        for sl_i in range(N_SLABS):
            jj = b * N_SLABS + sl_i
            s = slice(jj * JM, (jj + 1) * JM)
            sl = cen[b, sl_i * L_SLAB : (sl_i + 1) * L_SLAB]       # [16, 3]
            pf = sl[:, None, :] + PROBES[None].astype(np.float64)  # [16, 26, 3]
            mlf = pf.transpose(1, 0, 2).reshape(JM, 3)             # (m, l') major
            for k in range(D):
                p0, p1, p2 = _split3_bf16(8.0 * -20.0 * mlf[:, k])
                base = 6 * k
                p5[base + 0, s] = p0
                p5[base + 1, s] = p1
                p5[base + 2, s] = p0
                p5[base + 3, s] = p1
                p5[base + 4, s] = p0
                p5[base + 5, s] = p2
            r = 8.0 * (10.0 * (mlf**2).sum(-1) + 1.0)              # [JM] f64
            r0, r1, r2 = _split3_bf16(r)
            p5[18, s] = r0
            p5[19, s] = r1
            p5[20, s] = r2
            p5[21:24, s] = 8.0
    # replicate to row groups 0/32/64 for row-tiled matmul1
    p5r = np.zeros((96, N_JOBS * JM), bf16)
    for g in range(3):
        p5r[32 * g : 32 * g + K5, :] = p5
    return p5r


def _in_maps(points, centers, W_weighter):
    c5r, ft, wr = _prep_shared(points, W_weighter)
    zz = np.zeros((1, 128 + JM), bf16)
    return [
        {"c5": c5r, "ft": ft, "p5": _prep_probes5(centers, core), "wr": wr, "zz": zz}
        for core in range(N_CORES)
    ]


def kernel(points, centers, W_weighter, b_weighter):
    from concourse.bass_utils import run_bass_kernel_spmd

    points = np.asarray(points)
    centers = np.asarray(centers)
    W_weighter = np.asarray(W_weighter)
    b_weighter = np.asarray(b_weighter)

    nc = _get_nc()
    in_maps = _in_maps(points, centers, W_weighter)
    res = run_bass_kernel_spmd(nc, in_maps, core_ids=list(range(N_CORES))).results

    out = np.empty((B, L, OUT_D), np.float32)
    for core in range(N_CORES):
        rh = res[core]["out"]  # [128, OUT_D]: even-m half + odd-m half
        r = rh[0:64] + rh[64:128]
        for jj in range(N_JOBS):
            b, s = jj // N_SLABS, jj % N_SLABS
            lo = core * L_LOC + s * L_SLAB
            out[b, lo : lo + L_SLAB] = r[jj * L_SLAB : (jj + 1) * L_SLAB]
    out += b_weighter.astype(np.float32)[None, None, :]
    return out
